# revision 17
# baseline (speedup 1.0000x reference)
"""Embedding lookup (mixed const/trainable tables) on 8 Trainium2 NeuronCores.

Problem (full shapes, fp32):
    X          [524288, 128]   const table (only rows with const_mask==1 are read)
    const_mask [524288]        1 = const row (read from X), 0 = trainable row
    weight     [262144, 128]   trainable table, indexed by rank among mask==0 rows
    index      [262144]        lookup ids into the 524288-row id space
    out        [262144, 128]   out[i] = X[index[i]] if const else weight[var_pos[index[i]]]

Strategy (model parallel, deduplicated, bf16, cluster-covered):
    - Host compacts X to its const rows (Xe) so both tables have 262144 rows;
      both are row-sharded over the 8 cores (32768 rows/core/table so local
      row ids fit dma_gather's int16 index format).
    - Tables are converted to bf16 on host (round-to-nearest; max rel err
      2^-9 ~ 2e-3, well inside the 2e-2 gate). Halves HBM traffic and
      doubles the rows each DMA descriptor moves.
    - Each lookup routes to the owning (core, table) bucket and is
      DEDUPLICATED; duplicates expand in the host-side scatter. GPSIMD
      SWDGE descriptor generation (~9.3ns/descriptor, serialized on one Q7
      pair) is the limiting engine together with HBM (~358 GB/s r+w), so
      the covering balances descriptor count against junk traffic:
        * distinct sorted rows are merged into CLUSTERS across gaps <= G=2
          (junk rows inside a cluster are gathered and discarded on host)
        * each cluster of span L is covered by L//16 16-row (4KB)
          descriptors plus one tail descriptor of 2/4/8/16 rows that
          overlaps back into the cluster (or pads before it) so reads
          stay in-bounds.
      This cuts descriptors/core from ~16k to ~6.4k while coverage grows
      only ~1.9x (bf16 makes the bytes cheap).
    - Device kernel per core: sync engine (HWDGE) loads the packed index
      streams + counts while GPSIMD loads the ucode library; then 10
      dma_gather streams (X/W x tiers {16,8,4,2}; X16 split 128+896 so
      SDMA transfers start ~1.5us into gen, W2 split 768+256 so the
      kernel tail is one small write), each followed by a large HWDGE
      write SBUF->HBM, all overlapped. Exact counts ride in `cnts` and
      are loaded into Q7 registers so -1 index padding costs nothing.
    - Timeline per core (~115us vs 196us for the fp32 exact-cover
      predecessor): ~6.7us Tile preamble, ~9us ucode library reload,
      ~75us descriptor generation overlapped with ~26MB of HBM traffic
      at the ~358 GB/s per-NC cap (716 GB/s per 2-NC stack), small tail.
    - Host scatters the gathered distinct rows back to all lookup
      positions and upcasts to fp32.
"""

import numpy as np

import concourse.bass as bass
import concourse.bacc as bacc
import concourse.mybir as mybir
from concourse.bass_utils import run_bass_kernel_spmd
from concourse.library_config import mlp

NCORES = 8
D = 128             # feature dim; bf16 rows are 256B
SH = 32768          # table rows per core per table (int16 gather index limit)
G = 2               # merge clusters across gaps <= G rows
TIERS = (16, 8, 4, 2)

# Distinct rows per bucket: 16384 mean lookups hit ~12.9k distinct rows in
# ~7.8k runs; gap<=2 merging forms ~2.9k clusters covered by ~3.2k
# descriptors. Caps are rounded to 128 just above the per-bucket max (the
# input distribution is deterministic); buckets whose tier-8 tail count
# exceeds its cap promote the overflow to tier 16 (CAP8/REBIN16 below).
# X16 is split so SDMA transfers start after ~3us of gen instead of ~8;
# W2 is split so the kernel tail is one small write.
STREAMS = (
    ("X16a", "X", 16,  128, 0),
    ("X16b", "X", 16,  896, 128),
    ("W16",  "W", 16, 1024, 0),
    ("X8",   "X", 8,   768, 0),
    ("W8",   "W", 8,   768, 0),
    ("X4",   "X", 4,   640, 0),
    ("W4",   "W", 4,   640, 0),
    ("X2",   "X", 2,  1024, 0),
    ("W2a",  "W", 2,   768, 0),
    ("W2b",  "W", 2,   256, 768),
)
# per-tier descriptor-count caps (sum of the STREAMS caps above); tail
# descriptors overflowing a tier promote to the next tier up so static caps
# hold under any small drift of the input distribution
TIER_CAP = {2: 1024, 4: 640, 8: 768, 16: 1024}

_prog_cache = {}
LAST = {}  # debug/profiling introspection for test harnesses


def _build_program():
    """Per-core SPMD bass program: exact-count gather streams + writes."""
    nc = bacc.Bacc("TRN2", target_bir_lowering=False)

    tabs = {
        "X": nc.dram_tensor("tabX", [SH, D], mybir.dt.bfloat16, kind="ExternalInput"),
        "W": nc.dram_tensor("tabW", [SH, D], mybir.dt.bfloat16, kind="ExternalInput"),
    }
    total_cap16 = sum(cap for _, _, _, cap, _ in STREAMS) // 16
    idxall = nc.dram_tensor(
        "idxall", [128, total_cap16], mybir.dt.int16, kind="ExternalInput"
    )
    cnts = nc.dram_tensor(
        "cnts", [128, len(STREAMS)], mybir.dt.int32, kind="ExternalInput"
    )
    outs = {}
    for nm, b, t, cap, off in STREAMS:
        outs[nm] = nc.dram_tensor(
            f"out{nm}", [128, cap // 128, t * D], mybir.dt.bfloat16,
            kind="ExternalOutput",
        )

    from contextlib import ExitStack

    with ExitStack() as ctx:
        # write-completion sems already guarantee all DMAs retired; skipping
        # the gpsimd dge_drain removes ~10us from the kernel tail
        block = ctx.enter_context(nc.Block(no_gpsimd_drain=True))
        idx_sb = ctx.enter_context(
            nc.sbuf_tensor("isball", [128, total_cap16], mybir.dt.int16)
        )
        tiles, gsem, wsem = {}, {}, {}
        for nm, b, t, cap, off in STREAMS:
            tiles[nm] = ctx.enter_context(
                nc.sbuf_tensor(f"tile{nm}", [128, cap // 128, t * D],
                               mybir.dt.bfloat16)
            )
            gsem[nm] = ctx.enter_context(nc.semaphore(f"g{nm}"))
            wsem[nm] = ctx.enter_context(nc.semaphore(f"w{nm}"))
        csb = ctx.enter_context(
            nc.sbuf_tensor("csb", [128, len(STREAMS)], mybir.dt.int32)
        )
        io = ctx.enter_context(nc.semaphore("io"))

        # column offset of each stream's index block inside idxall
        col_off = {}
        acc = 0
        for nm, b, t, cap, off in STREAMS:
            col_off[nm] = acc
            acc += cap // 16

        @block.gpsimd
        def _(g: bass.BassGpSimd):
            # inputs arrive via sync/HWDGE; Q7 only needs the library first
            g.load_library(mlp)
            g.wait_ge(io, 32)
            from contextlib import ExitStack as ES

            with ES() as rctx:
                regs = {
                    nm: rctx.enter_context(g.register(f"r{nm}"))
                    for nm, *_ in STREAMS
                }
                for i, (nm, b, t, cap, off) in enumerate(STREAMS):
                    # reg_load just before each gather keeps the first
                    # transfer off the critical path
                    g.reg_load(regs[nm], csb[0:1, i : i + 1])
                    # overlapping view: row stride D, element t*D ->
                    # idx r reads rows r..r+t-1 as one descriptor
                    src = bass.AP(tabs[b], 0, [[D, SH - (t - 1)], [1, t * D]])
                    g.dma_gather(
                        tiles[nm][:],
                        src,
                        idx_sb[:, col_off[nm] : col_off[nm] + cap // 16],
                        cap,
                        regs[nm],
                        t * D,
                        elem_step=D,
                        single_packet=False,
                    ).then_inc(gsem[nm], 16)

        @block.sync
        def _(s: bass.BassEngine):
            s.dma_start(idx_sb[:], idxall[:]).then_inc(io, 16)
            s.dma_start(csb[:], cnts[:]).then_inc(io, 16)
            for nm, *_ in STREAMS:
                s.wait_ge(gsem[nm], 16)
                s.dma_start(outs[nm][:], tiles[nm][:]).then_inc(wsem[nm], 16)
            for nm, *_ in STREAMS:
                s.wait_ge(wsem[nm], 16)

    nc.compile()
    return nc


def get_program():
    if "nc" not in _prog_cache:
        _prog_cache["nc"] = _build_program()
    return _prog_cache["nc"]


def _slot_rows(cap):
    """Flattened [128*(cap/128), elem] device-buffer row per gather slot."""
    j = np.arange(cap, dtype=np.int64)
    return (j % 128) * (cap // 128) + j // 128


def _wrap_idx(seg, cap):
    """Pack a stream's int16 ids into the [128, cap/16] wrapped+replicated
    layout dma_gather expects (idx j at partition j%16, col j//16, replicated
    for the 8 Q7 cores), -1 padded."""
    pad = np.full(cap, -1, np.int16)
    pad[: seg.size] = seg
    wrapped = pad.reshape(cap // 16, 16).T  # [16, cap/16]
    return np.ascontiguousarray(np.tile(wrapped, (8, 1)))


def _to_bf16_u16(a):
    """fp32 -> bf16 bits with round-to-nearest-even."""
    u = np.ascontiguousarray(a, dtype=np.float32).view(np.uint32)
    r = u + 0x7FFF + ((u >> 16) & 1)
    return (r >> 16).astype(np.uint16)


def _route(cm, idx, n_weight_rows):
    """Deduplicated (bucket, local row) routing.

    Returns (ulocal, counts, inv, const_ids):
      ulocal    local table row per distinct slot, bucket-major, sorted
      counts    [16] distinct rows per bucket (bucket = slot*8 + core)
      inv       per-lookup index into the distinct-slot space
      const_ids row ids of X that form the compacted const table
    """
    const_rank = np.cumsum(cm) - 1
    var_pos = np.clip(np.cumsum(1 - cm) - 1, 0, n_weight_rows - 1)
    isc = cm[idx] > 0
    r = np.where(isc, const_rank[idx], var_pos[idx])
    bucket = (~isc).astype(np.int64) * NCORES + (r >> 15)
    key = bucket * SH + (r & (SH - 1))
    uniq, inv = np.unique(key, return_inverse=True)
    counts = np.bincount(uniq // SH, minlength=2 * NCORES)
    ulocal = uniq % SH
    const_ids = np.flatnonzero(cm > 0)
    return ulocal, counts, inv, const_ids


def _cover_clusters(u):
    """Cover sorted distinct rows with gap-merged multi-row descriptors.

    Rows are merged into clusters across gaps <= G; a cluster of span L is
    covered by L//16 16-row descriptors from the cluster start plus one tail
    descriptor (smallest tier >= remainder) that overlaps back into the
    cluster, clipped to table bounds.

    Returns (tiers, elmap):
      tiers = {T: sorted start rows} for T in TIERS
      elmap = (tier, start, off) arrays per element of u
    """
    n = u.size
    new = np.empty(n, bool)
    new[0] = True
    np.greater(np.diff(u), G + 1, out=new[1:])
    inew = np.flatnonzero(new)              # index into u of cluster starts
    cid = np.cumsum(new) - 1
    nrows = np.diff(np.append(inew, n))     # rows per cluster
    s = u[inew]                             # cluster start row
    e = u[inew + nrows - 1]                 # cluster end row
    span = e - s + 1
    n16 = span // 16
    t = span - 16 * n16
    # tail tier: smallest in TIERS >= t (0 if no tail)
    tt = np.select([t == 0, t <= 2, t <= 4, t <= 8], [0, 2, 4, 8], default=16)
    # promote overflowing tail tiers upward so static stream caps hold
    # (tier-16 overflow is caught by the caller's cap check instead)
    for T, nxt in ((2, 4), (4, 8), (8, 16)):
        nt = int((tt == T).sum())
        if nt > TIER_CAP[T]:
            tt[np.flatnonzero(tt == T)[: nt - TIER_CAP[T]]] = nxt
    q = np.clip(e + 1 - tt, 0, SH - np.maximum(tt, 1))  # tail desc start

    # interior 16-row descriptor starts, cluster-major (ascending)
    tot16 = int(n16.sum())
    base = np.repeat(s, n16)
    first = np.repeat(np.cumsum(n16) - n16, n16)
    int16s = base + 16 * (np.arange(tot16) - first)

    tiers = {}
    tiers[16] = np.sort(np.concatenate([int16s, q[tt == 16]]))
    for T in (8, 4, 2):
        tiers[T] = q[tt == T]               # ascending by construction

    # per-element mapping
    o = u - s[cid]                          # offset inside cluster span
    rn16 = n16[cid]
    in16 = o < 16 * rn16
    tier = np.empty(n, np.int64)
    start = np.empty(n, np.int64)
    tier[in16] = 16
    start[in16] = u[in16] - o[in16] % 16
    tl = ~in16
    tier[tl] = tt[cid[tl]]
    start[tl] = q[cid[tl]]
    off = u - start
    return tiers, (tier, start, off)


def _kernel_numpy(X, cm, weight, idx):
    """Host fallback (used only if structural assumptions break)."""
    var_pos = np.clip(np.cumsum(1 - cm) - 1, 0, weight.shape[0] - 1)
    isc = cm[idx] > 0
    out = np.where(isc[:, None], X[idx], weight[var_pos[idx]])
    return out.astype(np.float32)


def kernel(X, const_mask, weight, index):
    import ml_dtypes

    X = np.ascontiguousarray(np.asarray(X), dtype=np.float32)
    weight = np.ascontiguousarray(np.asarray(weight), dtype=np.float32)
    cm = np.asarray(const_mask).astype(np.int64)
    idx = np.asarray(index).astype(np.int64)

    ulocal, counts, inv, const_ids = _route(cm, idx, weight.shape[0])
    starts = np.concatenate([[0], np.cumsum(counts)])
    covers = [
        _cover_clusters(ulocal[starts[b] : starts[b + 1]]) if counts[b] else None
        for b in range(16)
    ]

    # per (bucket, tier): stream segments covering the id list
    segs = {}
    for nm, b, t, cap, off in STREAMS:
        segs.setdefault((b, t), []).append((nm, cap, off))

    def _cap_ok(bkt):
        if covers[bkt] is None:
            return False
        tiers, _ = covers[bkt]
        b = "X" if bkt < NCORES else "W"
        for t in TIERS:
            lst = segs[(b, t)]
            total_cap = sum(cap for _, cap, _ in lst)
            last_off = lst[-1][2]
            # every split segment must be non-empty (a zero-count gather is
            # undefined) and the full list must fit the combined capacity
            if not last_off < tiers[t].size <= total_cap:
                return False
        return True

    structural_ok = (
        X.shape == (524288, 128)
        and weight.shape == (262144, 128)
        and const_ids.size == NCORES * SH
        and weight.shape[0] == NCORES * SH
        and all(_cap_ok(bkt) for bkt in range(2 * NCORES))
    )
    if not structural_ok:
        return _kernel_numpy(X, cm, weight, idx)

    bf = ml_dtypes.bfloat16
    Xe16 = _to_bf16_u16(X[const_ids]).view(bf)   # compacted const table, bf16
    W16 = _to_bf16_u16(weight).view(bf)

    in_maps = []
    for c in range(NCORES):
        im = {
            "tabX": Xe16[c * SH : (c + 1) * SH],
            "tabW": W16[c * SH : (c + 1) * SH],
        }
        cvec = np.empty(len(STREAMS), np.int32)
        blocks = []
        for i, (nm, b, t, cap, off) in enumerate(STREAMS):
            bkt = (0 if b == "X" else NCORES) + c
            ids = covers[bkt][0][t][off : off + cap]
            blocks.append(_wrap_idx(ids.astype(np.int16), cap))
            cvec[i] = ids.size
        im["idxall"] = np.ascontiguousarray(np.hstack(blocks))
        im["cnts"] = np.ascontiguousarray(np.tile(cvec, (128, 1)))
        in_maps.append(im)

    nc = get_program()
    res = run_bass_kernel_spmd(nc, in_maps, core_ids=list(range(NCORES)))
    LAST["res"] = res

    # reassemble: distinct rows bucket-major, then expand duplicates per lookup
    allrows = np.empty((ulocal.size, D), np.uint16)
    for c in range(NCORES):
        for b in ("X", "W"):
            bkt = (0 if b == "X" else NCORES) + c
            tiers, (tier, start, off) = covers[bkt]
            seg = slice(starts[bkt], starts[bkt + 1])
            arr = np.empty((tier.size, D), np.uint16)
            for t in TIERS:
                m = tier == t
                pos = np.searchsorted(tiers[t], start[m])
                offm = off[m]
                vals = np.empty((pos.size, D), np.uint16)
                for snm, scap, soff in segs[(b, t)]:
                    buf = (
                        np.asarray(res.results[c][f"out{snm}"])
                        .view(np.uint16)
                        .reshape(-1, D)
                    )
                    sr = _slot_rows(scap)
                    sm = (pos >= soff) & (pos < soff + scap)
                    vals[sm] = buf[sr[pos[sm] - soff] * t + offm[sm]]
                arr[m] = vals
            allrows[seg] = arr
    out16 = allrows[inv]
    return (out16.astype(np.uint32) << 16).view(np.float32)


# revision 18
# speedup vs baseline: 1.0508x; 1.0508x over previous
"""Embedding lookup (mixed const/trainable tables) on 8 Trainium2 NeuronCores.

Problem (full shapes, fp32):
    X          [524288, 128]   const table (only rows with const_mask==1 are read)
    const_mask [524288]        1 = const row (read from X), 0 = trainable row
    weight     [262144, 128]   trainable table, indexed by rank among mask==0 rows
    index      [262144]        lookup ids into the 524288-row id space
    out        [262144, 128]   out[i] = X[index[i]] if const else weight[var_pos[index[i]]]

Strategy (model parallel, deduplicated, bf16, cluster-covered):
    - Host compacts X to its const rows (Xe) so both tables have 262144 rows;
      both are row-sharded over the 8 cores (32768 rows/core/table so local
      row ids fit dma_gather's int16 index format).
    - Tables are converted to bf16 on host (round-to-nearest; max rel err
      2^-9 ~ 2e-3, well inside the 2e-2 gate). Halves HBM traffic and
      doubles the rows each DMA descriptor moves.
    - Each lookup routes to the owning (core, table) bucket and is
      DEDUPLICATED; duplicates expand in the host-side scatter. GPSIMD
      SWDGE descriptor generation (~9.3ns/descriptor, serialized on one Q7
      pair) is the limiting engine together with HBM (~358 GB/s r+w), so
      the covering balances descriptor count against junk traffic:
        * distinct sorted rows are merged into CLUSTERS across gaps <= G=2
          (junk rows inside a cluster are gathered and discarded on host)
        * each cluster of span L is covered by L//16 16-row (4KB)
          descriptors plus one tail descriptor of 2/4/8/16 rows that
          overlaps back into the cluster (or pads before it) so reads
          stay in-bounds.
      This cuts descriptors/core from ~16k to ~6.4k while coverage grows
      only ~1.9x (bf16 makes the bytes cheap).
    - Device kernel per core: sync engine (HWDGE) loads the packed index
      streams + counts while GPSIMD loads the ucode library; then 10
      dma_gather streams (X/W x tiers {16,8,4,2}; X16 split 128+896 so
      SDMA transfers start ~1.5us into gen, W2 split 768+256 so the
      kernel tail is one small write), each followed by a large HWDGE
      write SBUF->HBM, all overlapped. Exact counts ride in `cnts` and
      are loaded into Q7 registers so -1 index padding costs nothing.
    - Timeline per core (~115us vs 196us for the fp32 exact-cover
      predecessor): ~6.7us Tile preamble, ~9us ucode library reload,
      ~75us descriptor generation overlapped with ~26MB of HBM traffic
      at the ~358 GB/s per-NC cap (716 GB/s per 2-NC stack), small tail.
    - Host scatters the gathered distinct rows back to all lookup
      positions and upcasts to fp32.
"""

import numpy as np

import concourse.bass as bass
import concourse.bacc as bacc
import concourse.mybir as mybir
from concourse.bass_utils import run_bass_kernel_spmd
from concourse.library_config import mlp

NCORES = 8
D = 128             # feature dim; bf16 rows are 256B
SH = 32768          # table rows per core per table (int16 gather index limit)
G = 2               # merge clusters across gaps <= G rows
TIERS = (16, 8, 4, 2)

# Distinct rows per bucket: 16384 mean lookups hit ~12.9k distinct rows in
# ~7.8k runs; gap<=2 merging forms ~2.9k clusters covered by ~3.2k
# descriptors. Caps are rounded to 128 just above the per-bucket max (the
# input distribution is deterministic); buckets whose tier-8 tail count
# exceeds its cap promote the overflow to tier 16 (CAP8/REBIN16 below).
# X16 is split so SDMA transfers start after ~3us of gen instead of ~8;
# W2 is split so the kernel tail is one small write.
STREAMS = (
    ("X16a", "X", 16,  128, 0),
    ("X16b", "X", 16,  384, 128),
    ("X16c", "X", 16,  512, 512),
    ("W16",  "W", 16, 1024, 0),
    ("X8",   "X", 8,   768, 0),
    ("W8",   "W", 8,   768, 0),
    ("X4",   "X", 4,   640, 0),
    ("W4",   "W", 4,   640, 0),
    ("X2",   "X", 2,  1024, 0),
    ("W2a",  "W", 2,   768, 0),
    ("W2b",  "W", 2,   256, 768),
)
# per-tier descriptor-count caps (sum of the STREAMS caps above); tail
# descriptors overflowing a tier promote to the next tier up so static caps
# hold under any small drift of the input distribution
TIER_CAP = {2: 1024, 4: 640, 8: 768, 16: 1024}

_prog_cache = {}
LAST = {}  # debug/profiling introspection for test harnesses


def _build_program():
    """Per-core SPMD bass program: exact-count gather streams + writes."""
    nc = bacc.Bacc("TRN2", target_bir_lowering=False)

    tabs = {
        "X": nc.dram_tensor("tabX", [SH, D], mybir.dt.bfloat16, kind="ExternalInput"),
        "W": nc.dram_tensor("tabW", [SH, D], mybir.dt.bfloat16, kind="ExternalInput"),
    }
    total_cap16 = sum(cap for _, _, _, cap, _ in STREAMS) // 16
    idxall = nc.dram_tensor(
        "idxall", [128, total_cap16], mybir.dt.int16, kind="ExternalInput"
    )
    cnts = nc.dram_tensor(
        "cnts", [128, len(STREAMS)], mybir.dt.int32, kind="ExternalInput"
    )
    outs = {}
    for nm, b, t, cap, off in STREAMS:
        outs[nm] = nc.dram_tensor(
            f"out{nm}", [128, cap // 128, t * D], mybir.dt.bfloat16,
            kind="ExternalOutput",
        )

    from contextlib import ExitStack

    with ExitStack() as ctx:
        # write-completion sems already guarantee all DMAs retired; skipping
        # the gpsimd dge_drain removes ~10us from the kernel tail
        block = ctx.enter_context(nc.Block(no_gpsimd_drain=True))
        idx_sb = ctx.enter_context(
            nc.sbuf_tensor("isball", [128, total_cap16], mybir.dt.int16)
        )
        tiles, gsem, wsem = {}, {}, {}
        for nm, b, t, cap, off in STREAMS:
            tiles[nm] = ctx.enter_context(
                nc.sbuf_tensor(f"tile{nm}", [128, cap // 128, t * D],
                               mybir.dt.bfloat16)
            )
            gsem[nm] = ctx.enter_context(nc.semaphore(f"g{nm}"))
            wsem[nm] = ctx.enter_context(nc.semaphore(f"w{nm}"))
        csb = ctx.enter_context(
            nc.sbuf_tensor("csb", [128, len(STREAMS)], mybir.dt.int32)
        )
        io = ctx.enter_context(nc.semaphore("io"))

        # column offset of each stream's index block inside idxall
        col_off = {}
        acc = 0
        for nm, b, t, cap, off in STREAMS:
            col_off[nm] = acc
            acc += cap // 16

        @block.gpsimd
        def _(g: bass.BassGpSimd):
            # inputs arrive via sync/HWDGE; Q7 only needs the library first
            g.load_library(mlp)
            g.wait_ge(io, 32)
            from contextlib import ExitStack as ES

            with ES() as rctx:
                regs = {
                    nm: rctx.enter_context(g.register(f"r{nm}"))
                    for nm, *_ in STREAMS
                }
                for i, (nm, b, t, cap, off) in enumerate(STREAMS):
                    # reg_load just before each gather keeps the first
                    # transfer off the critical path
                    g.reg_load(regs[nm], csb[0:1, i : i + 1])
                    # overlapping view: row stride D, element t*D ->
                    # idx r reads rows r..r+t-1 as one descriptor
                    src = bass.AP(tabs[b], 0, [[D, SH - (t - 1)], [1, t * D]])
                    g.dma_gather(
                        tiles[nm][:],
                        src,
                        idx_sb[:, col_off[nm] : col_off[nm] + cap // 16],
                        cap,
                        regs[nm],
                        t * D,
                        elem_step=D,
                        single_packet=False,
                    ).then_inc(gsem[nm], 16)

        @block.sync
        def _(s: bass.BassEngine):
            s.dma_start(idx_sb[:], idxall[:]).then_inc(io, 16)
            s.dma_start(csb[:], cnts[:]).then_inc(io, 16)
            for nm, *_ in STREAMS:
                s.wait_ge(gsem[nm], 16)
                s.dma_start(outs[nm][:], tiles[nm][:]).then_inc(wsem[nm], 16)
            for nm, *_ in STREAMS:
                s.wait_ge(wsem[nm], 16)

    nc.compile()
    return nc


def get_program():
    if "nc" not in _prog_cache:
        _prog_cache["nc"] = _build_program()
    return _prog_cache["nc"]


def _slot_rows(cap):
    """Flattened [128*(cap/128), elem] device-buffer row per gather slot."""
    j = np.arange(cap, dtype=np.int64)
    return (j % 128) * (cap // 128) + j // 128


def _wrap_idx(seg, cap):
    """Pack a stream's int16 ids into the [128, cap/16] wrapped+replicated
    layout dma_gather expects (idx j at partition j%16, col j//16, replicated
    for the 8 Q7 cores), -1 padded."""
    pad = np.full(cap, -1, np.int16)
    pad[: seg.size] = seg
    wrapped = pad.reshape(cap // 16, 16).T  # [16, cap/16]
    return np.ascontiguousarray(np.tile(wrapped, (8, 1)))


def _to_bf16_u16(a):
    """fp32 -> bf16 bits with round-to-nearest-even."""
    u = np.ascontiguousarray(a, dtype=np.float32).view(np.uint32)
    r = u + 0x7FFF + ((u >> 16) & 1)
    return (r >> 16).astype(np.uint16)


def _route(cm, idx, n_weight_rows):
    """Deduplicated (bucket, local row) routing.

    Returns (ulocal, counts, inv, const_ids):
      ulocal    local table row per distinct slot, bucket-major, sorted
      counts    [16] distinct rows per bucket (bucket = slot*8 + core)
      inv       per-lookup index into the distinct-slot space
      const_ids row ids of X that form the compacted const table
    """
    const_rank = np.cumsum(cm) - 1
    var_pos = np.clip(np.cumsum(1 - cm) - 1, 0, n_weight_rows - 1)
    isc = cm[idx] > 0
    r = np.where(isc, const_rank[idx], var_pos[idx])
    bucket = (~isc).astype(np.int64) * NCORES + (r >> 15)
    key = bucket * SH + (r & (SH - 1))
    uniq, inv = np.unique(key, return_inverse=True)
    counts = np.bincount(uniq // SH, minlength=2 * NCORES)
    ulocal = uniq % SH
    const_ids = np.flatnonzero(cm > 0)
    return ulocal, counts, inv, const_ids


def _cover_clusters(u):
    """Cover sorted distinct rows with gap-merged multi-row descriptors.

    Rows are merged into clusters across gaps <= G; a cluster of span L is
    covered by L//16 16-row descriptors from the cluster start plus one tail
    descriptor (smallest tier >= remainder) that overlaps back into the
    cluster, clipped to table bounds.

    Returns (tiers, elmap):
      tiers = {T: sorted start rows} for T in TIERS
      elmap = (tier, start, off) arrays per element of u
    """
    n = u.size
    new = np.empty(n, bool)
    new[0] = True
    np.greater(np.diff(u), G + 1, out=new[1:])
    inew = np.flatnonzero(new)              # index into u of cluster starts
    cid = np.cumsum(new) - 1
    nrows = np.diff(np.append(inew, n))     # rows per cluster
    s = u[inew]                             # cluster start row
    e = u[inew + nrows - 1]                 # cluster end row
    span = e - s + 1
    n16 = span // 16
    t = span - 16 * n16
    # tail tier: smallest in TIERS >= t (0 if no tail)
    tt = np.select([t == 0, t <= 2, t <= 4, t <= 8], [0, 2, 4, 8], default=16)
    # promote overflowing tail tiers upward so static stream caps hold
    # (tier-16 overflow is caught by the caller's cap check instead)
    for T, nxt in ((2, 4), (4, 8), (8, 16)):
        nt = int((tt == T).sum())
        if nt > TIER_CAP[T]:
            tt[np.flatnonzero(tt == T)[: nt - TIER_CAP[T]]] = nxt
    q = np.clip(e + 1 - tt, 0, SH - np.maximum(tt, 1))  # tail desc start

    # interior 16-row descriptor starts, cluster-major (ascending)
    tot16 = int(n16.sum())
    base = np.repeat(s, n16)
    first = np.repeat(np.cumsum(n16) - n16, n16)
    int16s = base + 16 * (np.arange(tot16) - first)

    tiers = {}
    tiers[16] = np.sort(np.concatenate([int16s, q[tt == 16]]))
    for T in (8, 4, 2):
        tiers[T] = q[tt == T]               # ascending by construction

    # per-element mapping
    o = u - s[cid]                          # offset inside cluster span
    rn16 = n16[cid]
    in16 = o < 16 * rn16
    tier = np.empty(n, np.int64)
    start = np.empty(n, np.int64)
    tier[in16] = 16
    start[in16] = u[in16] - o[in16] % 16
    tl = ~in16
    tier[tl] = tt[cid[tl]]
    start[tl] = q[cid[tl]]
    off = u - start
    return tiers, (tier, start, off)


def _kernel_numpy(X, cm, weight, idx):
    """Host fallback (used only if structural assumptions break)."""
    var_pos = np.clip(np.cumsum(1 - cm) - 1, 0, weight.shape[0] - 1)
    isc = cm[idx] > 0
    out = np.where(isc[:, None], X[idx], weight[var_pos[idx]])
    return out.astype(np.float32)


def kernel(X, const_mask, weight, index):
    import ml_dtypes

    X = np.ascontiguousarray(np.asarray(X), dtype=np.float32)
    weight = np.ascontiguousarray(np.asarray(weight), dtype=np.float32)
    cm = np.asarray(const_mask).astype(np.int64)
    idx = np.asarray(index).astype(np.int64)

    ulocal, counts, inv, const_ids = _route(cm, idx, weight.shape[0])
    starts = np.concatenate([[0], np.cumsum(counts)])
    covers = [
        _cover_clusters(ulocal[starts[b] : starts[b + 1]]) if counts[b] else None
        for b in range(16)
    ]

    # per (bucket, tier): stream segments covering the id list
    segs = {}
    for nm, b, t, cap, off in STREAMS:
        segs.setdefault((b, t), []).append((nm, cap, off))

    def _cap_ok(bkt):
        if covers[bkt] is None:
            return False
        tiers, _ = covers[bkt]
        b = "X" if bkt < NCORES else "W"
        for t in TIERS:
            lst = segs[(b, t)]
            total_cap = sum(cap for _, cap, _ in lst)
            last_off = lst[-1][2]
            # every split segment must be non-empty (a zero-count gather is
            # undefined) and the full list must fit the combined capacity
            if not last_off < tiers[t].size <= total_cap:
                return False
        return True

    structural_ok = (
        X.shape == (524288, 128)
        and weight.shape == (262144, 128)
        and const_ids.size == NCORES * SH
        and weight.shape[0] == NCORES * SH
        and all(_cap_ok(bkt) for bkt in range(2 * NCORES))
    )
    if not structural_ok:
        return _kernel_numpy(X, cm, weight, idx)

    bf = ml_dtypes.bfloat16
    Xe16 = _to_bf16_u16(X[const_ids]).view(bf)   # compacted const table, bf16
    W16 = _to_bf16_u16(weight).view(bf)

    in_maps = []
    for c in range(NCORES):
        im = {
            "tabX": Xe16[c * SH : (c + 1) * SH],
            "tabW": W16[c * SH : (c + 1) * SH],
        }
        cvec = np.empty(len(STREAMS), np.int32)
        blocks = []
        for i, (nm, b, t, cap, off) in enumerate(STREAMS):
            bkt = (0 if b == "X" else NCORES) + c
            ids = covers[bkt][0][t][off : off + cap]
            blocks.append(_wrap_idx(ids.astype(np.int16), cap))
            cvec[i] = ids.size
        im["idxall"] = np.ascontiguousarray(np.hstack(blocks))
        im["cnts"] = np.ascontiguousarray(np.tile(cvec, (128, 1)))
        in_maps.append(im)

    nc = get_program()
    res = run_bass_kernel_spmd(nc, in_maps, core_ids=list(range(NCORES)))
    LAST["res"] = res

    # reassemble: distinct rows bucket-major, then expand duplicates per lookup
    allrows = np.empty((ulocal.size, D), np.uint16)
    for c in range(NCORES):
        for b in ("X", "W"):
            bkt = (0 if b == "X" else NCORES) + c
            tiers, (tier, start, off) = covers[bkt]
            seg = slice(starts[bkt], starts[bkt + 1])
            arr = np.empty((tier.size, D), np.uint16)
            for t in TIERS:
                m = tier == t
                pos = np.searchsorted(tiers[t], start[m])
                offm = off[m]
                vals = np.empty((pos.size, D), np.uint16)
                for snm, scap, soff in segs[(b, t)]:
                    buf = (
                        np.asarray(res.results[c][f"out{snm}"])
                        .view(np.uint16)
                        .reshape(-1, D)
                    )
                    sr = _slot_rows(scap)
                    sm = (pos >= soff) & (pos < soff + scap)
                    vals[sm] = buf[sr[pos[sm] - soff] * t + offm[sm]]
                arr[m] = vals
            allrows[seg] = arr
    out16 = allrows[inv]
    return (out16.astype(np.uint32) << 16).view(np.float32)
